# revision 28
# baseline (speedup 1.0000x reference)
"""AttentionWithRoPE distributed Trainium2 kernel (8 NeuronCores), v3.

Sharding: pure 8-way tensor parallel over heads (2 heads = 128 hidden cols
per core), both batches on every core (seq concatenated to 4096 cols).
All activations stay transposed ([feature, seq]); no on-device transposes.

Design (driven by the observation that the PE mostly runs power/HAM
throttled at ~1.2 GHz, so PE cycles ~= wall clock):
  - Score matmuls: both heads packed at base partitions 0/64, K=64 each ->
    tile_position row-tiling runs them CONCURRENTLY (one 512-cycle pass).
  - Ctx matmuls: M=64 per head, col-tiled at output partitions 0/64 ->
    CONCURRENT (one 512-cycle pass, single [128,512] psum tile per block).
  - Softmax denominators: exp tiles are accumulated on the DVE (bf16 2x
    mode) into a per-block acc; one CONCURRENT pair of M=1 ones-matmuls
    (col positions 0/64) then yields both heads' denominators -> the old
    third matmul pass per ks step is gone (attention PE = 2 passes/ks).
  - Batch-pipelined: proj(b0) -> attention(b0) -> attention(b1), with
    proj(b1) statically interleaved into attention(b0)'s per-ks slots
    (biases/v-copies on GpSimd so the DVE/ACT stay clear).
  - Startup: input DMAs issued round-robin across four engine queues so
    issue latency doesn't serialize; x in [128,2048] chunks.
  - RoPE in bf16 (DVE 2x) writing straight into packed qr/kr tiles.
  - AllToAll covered by a right-sized dummy-matmul chain.
Bias folds (host): v-bias folds into output bias exactly; q pre-scaled by
1/sqrt(64) in its bias; compute dtype bf16 with fp32 PSUM accumulation.
"""

import numpy as np

HID = 1024
S = 2048
SB = 2 * S       # both batches, seq-concatenated
NHEAD = 16
D = 64
HPC = 2          # heads per core
OSL = 128        # hidden slice per core (HPC * D)
RB = 512         # global row block per core after AllToAll
NC = 8
ROPE_BASE = 10000.0

_cached = None
_last_in_maps = None


def _build_nc():
    import concourse.bacc as bacc
    import concourse.mybir as mybir
    from concourse import tile

    f32 = mybir.dt.float32
    bf16 = mybir.dt.bfloat16
    AF = mybir.ActivationFunctionType
    ALU = mybir.AluOpType

    nc = bacc.Bacc(None, target_bir_lowering=False)

    xT = nc.declare_dram_parameter("xT", [HID, SB], bf16, isOutput=False)
    wqT = nc.declare_dram_parameter("wqT", [HID, OSL], bf16, isOutput=False)
    wkT = nc.declare_dram_parameter("wkT", [HID, OSL], bf16, isOutput=False)
    wvT = nc.declare_dram_parameter("wvT", [HID, OSL], bf16, isOutput=False)
    woT = nc.declare_dram_parameter("woT", [HID, HID], bf16, isOutput=False)
    bqd = nc.declare_dram_parameter("bq", [128, 1], f32, isOutput=False)
    bkd = nc.declare_dram_parameter("bk", [128, 1], f32, isOutput=False)
    bod = nc.declare_dram_parameter("bo2", [128, 8], f32, isOutput=False)
    cosd = nc.declare_dram_parameter("cosT", [128, SB], bf16, isOutput=False)
    sind = nc.declare_dram_parameter("sinS", [128, SB], bf16, isOutput=False)
    out_ext = nc.declare_dram_parameter("out", [HID, RB], bf16, isOutput=True)

    a2a_in = nc.dram_tensor("a2a_in", [NC, OSL, RB], bf16)
    a2a_out = nc.dram_tensor("a2a_out", [NC, OSL, RB], bf16)

    NHC = HID // 128  # 8 hidden chunks

    with tile.TileContext(nc) as tc:
        with (
            tc.tile_pool(name="persist", bufs=1) as pp,
            tc.tile_pool(name="work", bufs=2) as wp,
            tc.tile_pool(name="exp", bufs=2) as ep,
            tc.tile_pool(name="ship", bufs=4) as sp,
        ):
            # ---------- input loads: spread issue over four engine queues ---
            def ptile(shape, dt_, tag):
                return pp.tile(shape, dt_, tag=tag, name=tag)

            wqb = [ptile([128, OSL], bf16, f"wqb{c}") for c in range(NHC)]
            wkb = [ptile([128, OSL], bf16, f"wkb{c}") for c in range(NHC)]
            wvb = [ptile([128, OSL], bf16, f"wvb{c}") for c in range(NHC)]
            xb = [[ptile([128, S], bf16, f"x{b}_{c}") for c in range(NHC)]
                  for b in range(2)]
            for c in range(NHC):
                nc.scalar.dma_start(out=wqb[c][:, :],
                                    in_=wqT[128 * c:128 * (c + 1), :])
                nc.sync.dma_start(out=xb[0][c][:, :],
                                  in_=xT[128 * c:128 * (c + 1), 0:S])
                nc.gpsimd.dma_start(out=wkb[c][:, :],
                                    in_=wkT[128 * c:128 * (c + 1), :])
                nc.gpsimd.dma_start(out=wvb[c][:, :],
                                    in_=wvT[128 * c:128 * (c + 1), :])
            bq_sb = ptile([128, 1], f32, "bq")
            bk_sb = ptile([128, 1], f32, "bk")
            bo_sb = ptile([128, 8], f32, "bo")
            nc.sync.dma_start(out=bq_sb[:, :], in_=bqd[:, :])
            nc.sync.dma_start(out=bk_sb[:, :], in_=bkd[:, :])
            nc.sync.dma_start(out=bo_sb[:, :], in_=bod[:, :])
            cos_sb = ptile([128, SB], bf16, "cos")
            sin_sb = ptile([128, SB], bf16, "sin")
            nc.scalar.dma_start(out=cos_sb[:, :], in_=cosd[:, :])
            nc.scalar.dma_start(out=sin_sb[:, :], in_=sind[:, :])
            for c in range(NHC):
                nc.scalar.dma_start(out=xb[1][c][:, :],
                                    in_=xT[128 * c:128 * (c + 1), S:SB])
            wob = [ptile([128, HID], bf16, f"wob{c}") for c in range(NHC)]
            for c in range(NHC):
                nc.sync.dma_start(out=wob[c][:, :],
                                  in_=woT[128 * c:128 * (c + 1), :])

            # ---------- persistent activation tiles ----------
            qsb = pp.tile([128, SB], bf16, tag="qsb", name="qsb")
            ksb = pp.tile([128, SB], bf16, tag="ksb", name="ksb")
            qr = pp.tile([128, SB], bf16, tag="qr", name="qr")
            kr = pp.tile([128, SB], bf16, tag="kr", name="kr")
            vsb = [pp.tile([128, 128], bf16, tag=f"vsb{st}", name=f"vsb{st}")
                   for st in range(SB // 128)]
            ones2 = pp.tile([128, 2], bf16, tag="ones2", name="ones2")
            nc.gpsimd.memset(ones2[:, :], 1.0)

            # ---------- PSUM pools (8 banks: 4 + 3 + 1) ----------
            _cmS = tc.tile_pool(name="psS", bufs=2, space="PSUM")
            _cmC = tc.tile_pool(name="psC", bufs=3, space="PSUM")
            _cmB = tc.tile_pool(name="psB1", bufs=1, space="PSUM")
            psS = _cmS.__enter__()
            psC = _cmC.__enter__()
            psB1 = _cmB.__enter__()

            # ---------- helpers ----------
            def rope_quarter(src, dest, q4, dma_eng):
                sl = slice(1024 * q4, 1024 * (q4 + 1))
                qswp = wp.tile([128, 1024], bf16, tag="qswp")
                for blk in range(4):
                    dlo = 32 * blk
                    srow = 32 * (blk + 1) if blk % 2 == 0 else 32 * (blk - 1)
                    dma_eng.dma_start(
                        out=qswp[dlo:dlo + 32, :],
                        in_=src[srow:srow + 32, sl])
                t1 = wp.tile([128, 1024], bf16, tag="ropet1")
                t2 = wp.tile([128, 1024], bf16, tag="ropet2")
                nc.vector.tensor_mul(t1[:, :], src[:, sl], cos_sb[:, sl])
                nc.vector.tensor_mul(t2[:, :], qswp[:, :], sin_sb[:, sl])
                nc.vector.tensor_add(dest[:, sl], t1[:, :], t2[:, :])

            # ---------- phase 1: proj + rope for batch 0 (ACT biases) ------
            for g in range(4):          # 512-col groups within batch 0
                cols = slice(512 * g, 512 * (g + 1))
                ps = psS.tile([128, 1024], f32, tag="mm1024")
                for c in range(NHC):
                    nc.tensor.matmul(
                        ps[:, 0:512], lhsT=wqb[c][:, :], rhs=xb[0][c][:, cols],
                        start=(c == 0), stop=(c == NHC - 1))
                    nc.tensor.matmul(
                        ps[:, 512:1024], lhsT=wkb[c][:, :],
                        rhs=xb[0][c][:, cols],
                        start=(c == 0), stop=(c == NHC - 1))
                nc.scalar.activation(qsb[:, cols], ps[:, 0:512], AF.Identity,
                                     bias=bq_sb[:, 0:1], scale=0.125)
                nc.scalar.activation(ksb[:, cols], ps[:, 512:1024],
                                     AF.Identity, bias=bk_sb[:, 0:1],
                                     scale=1.0)
                for st4 in range(4):
                    vps = psC.tile([128, 128], f32, tag="acc",
                                   padded_shape=[128, 512])
                    for c in range(NHC):
                        nc.tensor.matmul(
                            vps[:, :],
                            lhsT=xb[0][c][:, 512 * g + 128 * st4:
                                          512 * g + 128 * (st4 + 1)],
                            rhs=wvb[c][:, :],
                            start=(c == 0), stop=(c == NHC - 1))
                    nc.scalar.activation(vsb[4 * g + st4][:, :], vps[:, :],
                                         AF.Copy)
                if g % 2 == 1:
                    rope_quarter(qsb, qr, g // 2, nc.gpsimd)
                    rope_quarter(ksb, kr, g // 2, nc.gpsimd)

            # ---------- proj(b1) work units, interleaved into attention(b0) -
            b1_slots = []

            def _qk_group(wb, bias, scale, dest, cols, cell, bat, coff):
                def mk(c0, c1, finish):
                    def emit():
                        if c0 == 0:
                            cell[0] = psB1.tile([128, 512], f32, tag="pb1",
                                                name="pb1")
                        ps = cell[0]
                        for c in range(c0, c1):
                            nc.tensor.matmul(
                                ps[:, :], lhsT=wb[c][:, :],
                                rhs=xb[bat][c][:, cols],
                                start=(c == 0), stop=(c == NHC - 1))
                        if finish:
                            nc.vector.tensor_scalar(
                                dest[:, coff + cols.start:coff + cols.stop],
                                ps[:, :], scale, bias[:, 0:1],
                                ALU.mult, ALU.add)
                    return emit
                return [mk(0, 2, False), mk(2, 4, False), mk(4, 6, False),
                        mk(6, 8, True)]

            def _v_block(st, cols, cell):
                def mk(c0, c1, finish):
                    def emit():
                        if c0 == 0:
                            cell[0] = psB1.tile([128, 128], f32, tag="pb1",
                                                name="pb1",
                                                padded_shape=[128, 512])
                        ps = cell[0]
                        for c in range(c0, c1):
                            nc.tensor.matmul(
                                ps[:, :], lhsT=xb[1][c][:, cols],
                                rhs=wvb[c][:, :],
                                start=(c == 0), stop=(c == NHC - 1))
                        if finish:
                            nc.vector.tensor_copy(vsb[S // 128 + st][:, :],
                                                  ps[:, :])
                    return emit
                return [mk(0, 4, False), mk(4, 8, True)]

            for g in range(4):
                cols = slice(512 * g, 512 * (g + 1))
                qcell, kcell = [None], [None]
                b1_slots += _qk_group(wqb, bq_sb, 0.125, qsb, cols, qcell,
                                      1, S)
                b1_slots += _qk_group(wkb, bk_sb, 1.0, ksb, cols, kcell,
                                      1, S)
                for st4 in range(4):
                    vcell = [None]
                    b1_slots += _v_block(4 * g + st4,
                                         slice(512 * g + 128 * st4,
                                               512 * g + 128 * (st4 + 1)),
                                         vcell)
                if g % 2 == 1:
                    q4 = 2 + g // 2

                    def mk_rope(q4):
                        def emit():
                            rope_quarter(qsb, qr, q4, nc.gpsimd)
                            rope_quarter(ksb, kr, q4, nc.gpsimd)
                        return emit
                    b1_slots.append(mk_rope(q4))

            # ---------- attention ----------
            last_ct = [None]
            pend_epi = []   # stage closures of the previous block's epilogue

            def attention(b, slots):
                pulled = [0]
                total = len(slots)

                def pull(t, nt):
                    if total == 0:
                        return
                    want = ((t + 1) * total) // nt
                    while pulled[0] < want:
                        slots[pulled[0]]()
                        pulled[0] += 1

                for qs in range(4):
                    q0 = S * b + 512 * qs
                    cps = psC.tile([128, 512], f32, tag="acc")
                    # two half-accumulators (8 bf16 adds each) keep the
                    # denominator rounding error down; halves are summed
                    # exactly in PSUM by accumulating the ones-matmuls.
                    acc0 = ep.tile([128, 1024], bf16, tag="acc_et0", bufs=2)
                    acc1 = ep.tile([128, 1024], bf16, tag="acc_et1", bufs=2)
                    accs = (acc0, acc1)
                    sps_l = {}

                    def scores(ks):
                        k0 = S * b + 128 * ks
                        sps = psS.tile([128, 1024], f32, tag="mm1024")
                        nc.tensor.matmul(
                            sps[:, 0:512], lhsT=kr[0:64, k0:k0 + 128],
                            rhs=qr[0:64, q0:q0 + 512],
                            start=True, stop=True)
                        nc.tensor.matmul(
                            sps[:, 512:1024], lhsT=kr[64:128, k0:k0 + 128],
                            rhs=qr[64:128, q0:q0 + 512],
                            start=True, stop=True)
                        sps_l[ks] = sps

                    scores(0)
                    scores(1)
                    for ks in range(16):
                        et = ep.tile([128, 1024], bf16, tag="expT", bufs=4)
                        nc.scalar.activation(et[:, :], sps_l.pop(ks)[:, :],
                                             AF.Exp)
                        if ks < 14:
                            scores(ks + 2)
                        vt = vsb[(S * b) // 128 + ks]
                        # col-tiled concurrent ctx pair (out partitions 0/64)
                        nc.tensor.matmul(
                            cps[0:64, :], lhsT=vt[:, 0:64],
                            rhs=et[:, 0:512],
                            start=(ks == 0), stop=(ks == 15))
                        nc.tensor.matmul(
                            cps[64:128, :], lhsT=vt[:, 64:128],
                            rhs=et[:, 512:1024],
                            start=(ks == 0), stop=(ks == 15))
                        # DVE-accumulated exp sum (feeds the denominator mms)
                        a = accs[ks // 8]
                        if ks % 8 == 0:
                            nc.vector.tensor_copy(a[:, :], et[:, :])
                        else:
                            nc.vector.tensor_add(a[:, :], a[:, :], et[:, :])
                        # previous block's epilogue dribbles out one stage
                        # per even ks so it never bursts the PE/DVE queues
                        if pend_epi and ks % 2 == 0:
                            pend_epi.pop(0)()
                        pull(16 * qs + ks, 64)

                    def mk_epi(cps=cps, accs=accs, b=b, qs=qs):
                        st = {}

                        def s1():
                            # denominators: concurrent M=1 ones-matmul
                            # pairs; half-accumulators summed in PSUM
                            dps = psC.tile([128, 512], f32, tag="acc",
                                           name="dps")
                            for ia, a in enumerate(accs):
                                nc.tensor.matmul(
                                    dps[0:1, :], lhsT=ones2[:, 0:1],
                                    rhs=a[:, 0:512],
                                    start=(ia == 0), stop=(ia == 1))
                                nc.tensor.matmul(
                                    dps[64:65, :], lhsT=ones2[:, 1:2],
                                    rhs=a[:, 512:1024],
                                    start=(ia == 0), stop=(ia == 1))
                            st["dps"] = dps

                        def s2():
                            dps = st["dps"]
                            rec0 = ep.tile([1, 512], f32, tag="rec0",
                                           name="rec0")
                            nc.vector.reciprocal(rec0[:, :], dps[0:1, :])
                            rec64 = ep.tile([65, 512], f32, tag="rec64",
                                            name="rec64")
                            nc.vector.reciprocal(rec64[64:65, :],
                                                 dps[64:65, :])
                            st["rec0"], st["rec64"] = rec0, rec64

                        def s3():
                            hopt = ep.tile([1, 512], f32, tag="hopt",
                                           name="hopt")
                            nc.gpsimd.dma_start(out=hopt[:, :],
                                                in_=st["rec64"][64:65, :])
                            rb = ep.tile([128, 512], f32, tag="recb",
                                         name="recb")
                            rbt = ep.tile([64, 512], f32, tag="recbt",
                                          name="recbt")
                            nc.gpsimd.partition_broadcast(rb[0:64, :],
                                                          st["rec0"][:, :])
                            nc.gpsimd.partition_broadcast(rbt[:, :],
                                                          hopt[:, :])
                            nc.gpsimd.dma_start(out=rb[64:128, :],
                                                in_=rbt[:, :])
                            st["rb"] = rb

                        def s4():
                            ct = sp.tile([128, 512], bf16, tag="ct",
                                         name="ct")
                            nc.vector.tensor_mul(ct[:, :], cps[:, :],
                                                 st["rb"][:, :])
                            nc.sync.dma_start(out=a2a_in[4 * b + qs, :, :],
                                              in_=ct[:, :])
                            if b == 1 and qs == 3:
                                last_ct[0] = ct
                        return [s1, s2, s3, s4]
                    pend_epi.extend(mk_epi())

            attention(0, b1_slots)
            attention(1, [])
            while pend_epi:
                pend_epi.pop(0)()
            _cmB.__exit__(None, None, None)
            _cmC.__exit__(None, None, None)
            _cmS.__exit__(None, None, None)

            # ---------- AllToAll ----------
            nc.gpsimd.collective_compute(
                "AllToAll", mybir.AluOpType.bypass,
                replica_groups=[list(range(NC))],
                ins=[a2a_in.ap().opt()],
                outs=[a2a_out.ap().opt()])

            # ---------- output projection ----------
            _cmO = tc.tile_pool(name="psO", bufs=1, space="PSUM")
            psO = _cmO.__enter__()

            dumsrc = pp.tile([128, 512], bf16, tag="dumsrc")
            nc.gpsimd.memset(dumsrc[:, :], 0.0)
            nc.vector.tensor_copy(dumsrc[0:64, :], last_ct[0][0:64, :])
            dum = None
            for i in range(70):
                dum = psO.tile([128, 512], f32, tag="dum", bufs=2, name="dum")
                nc.tensor.matmul(
                    dum[:, :], lhsT=wob[0][:, 0:128],
                    rhs=dumsrc[:, :], start=True, stop=True)
            dumr = ep.tile([128, 128], f32, tag="dumr")
            nc.vector.tensor_copy(dumr[:, :], dum[:, 0:128])
            dead = nc.dram_tensor("dead", [128, 128], f32)
            nc.sync.dma_start(out=dead[:, :], in_=dumr[:, :])

            cxs = []
            for c in range(NHC):
                cx = pp.tile([128, RB], bf16, tag=f"cxb{c}", name=f"cxb{c}")
                nc.sync.dma_start(out=cx[:, :], in_=a2a_out[c, :, :])
                cxs.append(cx)
            for ot in range(8):
                ops = psO.tile([128, 512], f32, tag="ops", bufs=4)
                for c in range(NHC):
                    nc.tensor.matmul(
                        ops[:, :],
                        lhsT=wob[c][:, 128 * ot:128 * (ot + 1)],
                        rhs=cxs[c][:, :],
                        start=(c == 0), stop=(c == NHC - 1))
                osb = ep.tile([128, RB], bf16, tag="osb", bufs=3)
                nc.scalar.activation(
                    osb[:, :], ops[:, :], AF.Identity,
                    bias=bo_sb[:, ot:ot + 1], scale=1.0)
                nc.sync.dma_start(
                    out=out_ext[128 * ot:128 * (ot + 1), :], in_=osb[:, :])
            _cmO.__exit__(None, None, None)

    nc.finalize()
    return nc


def _host_tables():
    inv = 1.0 / (ROPE_BASE ** (np.arange(0, D, 2, dtype=np.float64) / D))
    pos = np.arange(S, dtype=np.float64)
    freqs = np.outer(pos, inv)                      # [S, 32]
    emb = np.concatenate([freqs, freqs], axis=-1)   # [S, 64]
    cosT = np.cos(emb).T.astype(np.float32)         # [64, S]
    sinT = np.sin(emb).T.astype(np.float32)
    sinS = np.concatenate([-sinT[:32], sinT[32:]], axis=0)
    cos2 = np.ascontiguousarray(np.tile(cosT, (2, 2)))   # [128, 2S]
    sin2 = np.ascontiguousarray(np.tile(sinS, (2, 2)))
    return cos2, sin2


def kernel(**inputs):
    import ml_dtypes
    from concourse.bass_utils import run_bass_kernel_spmd

    global _cached, _last_in_maps
    if _cached is None:
        _cached = _build_nc()
    nc = _cached

    bf = ml_dtypes.bfloat16
    hs = np.asarray(inputs["hidden_states"], dtype=np.float32)
    Wq = np.asarray(inputs["Wq"], dtype=np.float32)
    bq = np.asarray(inputs["bq"], dtype=np.float32)
    Wk = np.asarray(inputs["Wk"], dtype=np.float32)
    bk = np.asarray(inputs["bk"], dtype=np.float32)
    Wv = np.asarray(inputs["Wv"], dtype=np.float32)
    bv = np.asarray(inputs["bv"], dtype=np.float32)
    Wo = np.asarray(inputs["Wo"], dtype=np.float32)
    bo = np.asarray(inputs["bo"], dtype=np.float32)

    cos2, sin2 = _host_tables()
    cos2 = cos2.astype(bf)
    sin2 = sin2.astype(bf)
    bo2 = bo + bv @ Wo.T                                 # fold v-bias exactly
    bo2m = np.ascontiguousarray(bo2.reshape(8, 128).T)   # [128, 8]
    xTfull = np.ascontiguousarray(
        np.concatenate([hs[0].T, hs[1].T], axis=1)).astype(bf)  # [1024, 4096]
    woTc = np.ascontiguousarray(Wo.T).astype(bf)

    in_maps = []
    for c in range(NC):
        sl = slice(OSL * c, OSL * (c + 1))
        in_maps.append({
            "xT": xTfull,
            "wqT": np.ascontiguousarray(Wq[sl, :].T).astype(bf),
            "wkT": np.ascontiguousarray(Wk[sl, :].T).astype(bf),
            "wvT": np.ascontiguousarray(Wv[sl, :].T).astype(bf),
            "woT": woTc,
            "bq": np.ascontiguousarray((bq[sl] * 0.125)[:, None]),
            "bk": np.ascontiguousarray(bk[sl][:, None]),
            "bo2": bo2m,
            "cosT": cos2,
            "sinS": sin2,
        })

    _last_in_maps = in_maps
    res = run_bass_kernel_spmd(nc, in_maps, core_ids=list(range(NC)))
    out = np.empty((2, S, HID), dtype=np.float32)
    for c in range(NC):
        b, g = divmod(c, 4)
        out[b, RB * g:RB * (g + 1), :] = res.results[c]["out"].T.astype(np.float32)
    return out
